# revision 1
# baseline (speedup 1.0000x reference)
"""Additive attention scorer: S[b,q,k] = sum_h wv[h] * tanh((qs@Wq)[b,q,h] + (ks@Wk)[b,k,h]).

Sharding: data-parallel over batch B=8 across the 8 NeuronCores (one batch
element per core). Per core:
  1. PE projects qT = Wq^T @ qs[b]^T and kT = Wk^T @ ks[b]^T  -> [H=128, 512]
     (H lands exactly on the 128 partitions).
  2. For each query row lq: DVE builds X = kT + qT[:, lq] (per-partition
     scalar broadcast add), batched J rows per SBUF tile.
  3. ACT applies one big tanh over the batched tile (amortizes the ~224-cycle
     per-instruction overhead).
  4. PE reduces over H with a shifted-wv stationary trick: lhsT is a
     [128,128] window of a [128,255] tensor holding wv in column 127, so the
     single nonzero output row of each matmul lands on PSUM partition
     (lq mod 128); 128 matmuls accumulate one [128,512] output block.
  5. DVE copies PSUM->SBUF, DMA to DRAM.
"""

import numpy as np

B, LQ, LK, D, H = 8, 512, 512, 512, 128
P = 128           # SBUF partitions
ND = D // P       # contraction chunks for the projections
J = 8             # query rows batched per ACT instruction

_cache = {}


def _build():
    import concourse.bass as bass
    import concourse.tile as tile
    from concourse import bacc, mybir

    f32 = mybir.dt.float32
    nc = bacc.Bacc("TRN2", target_bir_lowering=False, debug=False, num_devices=B)

    qsT = nc.dram_tensor("qsT", [D, LQ], f32, kind="ExternalInput")
    ksT = nc.dram_tensor("ksT", [D, LK], f32, kind="ExternalInput")
    Wq = nc.dram_tensor("Wq", [D, H], f32, kind="ExternalInput")
    Wk = nc.dram_tensor("Wk", [D, H], f32, kind="ExternalInput")
    wbig = nc.dram_tensor("wbig", [H, 2 * H - 1], f32, kind="ExternalInput")
    out = nc.dram_tensor("out", [LQ, LK], f32, kind="ExternalOutput")

    with tile.TileContext(nc) as tc:
        with (
            tc.tile_pool(name="const", bufs=1) as cpool,
            tc.tile_pool(name="load", bufs=ND) as lpool,
            tc.tile_pool(name="x", bufs=3) as xpool,
            tc.tile_pool(name="y", bufs=2) as ypool,
            tc.tile_pool(name="sout", bufs=2) as spool,
            tc.tile_pool(name="ppsum", bufs=2, space=bass.MemorySpace.PSUM) as ppool,
            tc.tile_pool(name="spsum", bufs=2, space=bass.MemorySpace.PSUM) as sppool,
        ):
            wb = cpool.tile([P, 2 * H - 1], f32, tag="wb")
            nc.sync.dma_start(wb[:], wbig[:])

            # Projections: dst = W^T @ srcT accumulated over ND chunks of D.
            qT = cpool.tile([H, LQ], f32, tag="qT")
            kT = cpool.tile([H, LK], f32, tag="kT")
            for src, w_dram, dst in ((qsT, Wq, qT), (ksT, Wk, kT)):
                ps = ppool.tile([H, LQ], f32, tag="proj")
                for c in range(ND):
                    w_sb = lpool.tile([P, H], f32, tag="wchunk")
                    nc.sync.dma_start(w_sb[:], w_dram[c * P : (c + 1) * P, :])
                    x_sb = lpool.tile([P, LQ], f32, tag="xchunk")
                    nc.sync.dma_start(x_sb[:], src[c * P : (c + 1) * P, :])
                    nc.tensor.matmul(
                        ps[:], w_sb[:], x_sb[:], start=(c == 0), stop=(c == ND - 1)
                    )
                nc.vector.tensor_copy(dst[:], ps[:])

            tanh = mybir.ActivationFunctionType.Tanh
            NBLK = LQ // P  # output row-blocks
            GPB = P // J    # groups per block
            for blk in range(NBLK):
                sp = sppool.tile([P, LK], f32, tag="spsum")
                for g in range(GPB):
                    x = xpool.tile([P, J * LK], f32, tag="x")
                    for j in range(J):
                        lq = blk * P + g * J + j
                        nc.vector.tensor_scalar_add(
                            x[:, j * LK : (j + 1) * LK], kT[:], qT[:, lq : lq + 1]
                        )
                    y = ypool.tile([P, J * LK], f32, tag="y")
                    nc.scalar.activation(y[:], x[:], tanh)
                    for j in range(J):
                        r = g * J + j
                        nc.tensor.matmul(
                            sp[:],
                            wb[:, H - 1 - r : 2 * H - 1 - r],
                            y[:, j * LK : (j + 1) * LK],
                            start=(r == 0),
                            stop=(r == P - 1),
                        )
                s_sb = spool.tile([P, LK], f32, tag="sout")
                nc.vector.tensor_copy(s_sb[:], sp[:])
                nc.sync.dma_start(out[blk * P : (blk + 1) * P, :], s_sb[:])

    nc.compile()
    return nc


def _in_maps(qs, ks, Wq, Wk, wv):
    wbig = np.zeros((H, 2 * H - 1), np.float32)
    wbig[:, H - 1] = wv
    maps = []
    for b in range(B):
        maps.append(
            {
                "qsT": np.ascontiguousarray(np.asarray(qs)[b].T, dtype=np.float32),
                "ksT": np.ascontiguousarray(np.asarray(ks)[b].T, dtype=np.float32),
                "Wq": np.ascontiguousarray(Wq, dtype=np.float32),
                "Wk": np.ascontiguousarray(Wk, dtype=np.float32),
                "wbig": wbig,
            }
        )
    return maps


def run(qs, ks, Wq, Wk, wv, trace=False):
    from concourse.bass_utils import run_bass_kernel_spmd

    if "nc" not in _cache:
        _cache["nc"] = _build()
    res = run_bass_kernel_spmd(
        _cache["nc"],
        _in_maps(qs, ks, Wq, Wk, wv),
        core_ids=list(range(B)),
        trace=trace,
    )
    outs = np.stack([np.asarray(res.results[i]["out"]) for i in range(B)], axis=0)
    return outs.astype(np.float32), res


def kernel(qs, ks, Wq, Wk, wv):
    out, _ = run(qs, ks, Wq, Wk, wv, trace=False)
    return out


# revision 2
# speedup vs baseline: 1.5361x; 1.5361x over previous
"""Additive attention scorer: S[b,q,k] = sum_h wv[h] * tanh((qs@Wq)[b,q,h] + (ks@Wk)[b,k,h]).

Sharding: data-parallel over batch B=8 across the 8 NeuronCores (one batch
element per core). Per core:
  1. PE projects qT = Wq^T @ qs[b]^T and kT = Wk^T @ ks[b]^T  -> [H=128, 512]
     (H lands exactly on the 128 partitions). Inputs come in bf16 (the
     tanh-argument error budget tolerates it; halves DMA and keeps the
     matmuls single-pass -- fp32 matmuls decompose into 2 HW passes).
  2. For each query row lq: DVE builds X = kT + qT[:, lq] (per-partition
     scalar broadcast add) in bf16 (4x DVE mode), batched J rows per tile.
  3. ACT applies one big tanh over the batched tile (amortizes the ~224-cycle
     per-instruction overhead; ACT rate is dtype-independent so it is the
     hard bottleneck at ~1 elem/cycle/lane).
  4. PE reduces over H with a shifted-wv stationary trick: lhsT is a
     [128,128] window of a [128,255] bf16 tensor holding wv in column 127,
     so the single nonzero output row of each matmul lands on PSUM partition
     (lq mod 128); 128 matmuls accumulate one [128,512] fp32 output block.
  5. DVE copies PSUM->SBUF, DMA to DRAM.
"""

import numpy as np

B, LQ, LK, D, H = 8, 512, 512, 512, 128
P = 128           # SBUF partitions
ND = D // P       # contraction chunks for the projections
J = 16            # query rows batched per ACT instruction

_cache = {}


def _build():
    import concourse.bass as bass
    import concourse.tile as tile
    from concourse import bacc, mybir

    f32 = mybir.dt.float32
    bf16 = mybir.dt.bfloat16
    nc = bacc.Bacc("TRN2", target_bir_lowering=False, debug=False, num_devices=B)

    qsT = nc.dram_tensor("qsT", [D, LQ], bf16, kind="ExternalInput")
    ksT = nc.dram_tensor("ksT", [D, LK], bf16, kind="ExternalInput")
    Wq = nc.dram_tensor("Wq", [D, H], bf16, kind="ExternalInput")
    Wk = nc.dram_tensor("Wk", [D, H], bf16, kind="ExternalInput")
    wbig = nc.dram_tensor("wbig", [H, 2 * H - 1], bf16, kind="ExternalInput")
    out = nc.dram_tensor("out", [LQ, LK], f32, kind="ExternalOutput")

    with tile.TileContext(nc) as tc:
        with (
            tc.tile_pool(name="const", bufs=1) as cpool,
            tc.tile_pool(name="load", bufs=ND) as lpool,
            tc.tile_pool(name="x", bufs=3) as xpool,
            tc.tile_pool(name="y", bufs=2) as ypool,
            tc.tile_pool(name="sout", bufs=2) as spool,
            tc.tile_pool(name="ppsum", bufs=2, space=bass.MemorySpace.PSUM) as ppool,
            tc.tile_pool(name="spsum", bufs=2, space=bass.MemorySpace.PSUM) as sppool,
        ):
            wb = cpool.tile([P, 2 * H - 1], bf16, tag="wb")
            nc.sync.dma_start(wb[:], wbig[:])

            # Projections: dst = W^T @ srcT accumulated over ND chunks of D.
            # kT is stored bf16 (streamed operand of the adds); qT stays fp32
            # (per-partition scalar operand, exempt from DVE mode rules).
            qT = cpool.tile([H, LQ], f32, tag="qT")
            kT = cpool.tile([H, LK], bf16, tag="kT")
            for src, w_dram, dst in ((qsT, Wq, qT), (ksT, Wk, kT)):
                ps = ppool.tile([H, LQ], f32, tag="proj")
                for c in range(ND):
                    w_sb = lpool.tile([P, H], bf16, tag="wchunk")
                    nc.sync.dma_start(w_sb[:], w_dram[c * P : (c + 1) * P, :])
                    x_sb = lpool.tile([P, LQ], bf16, tag="xchunk")
                    nc.sync.dma_start(x_sb[:], src[c * P : (c + 1) * P, :])
                    nc.tensor.matmul(
                        ps[:], w_sb[:], x_sb[:], start=(c == 0), stop=(c == ND - 1)
                    )
                nc.vector.tensor_copy(dst[:], ps[:])

            tanh = mybir.ActivationFunctionType.Tanh
            NBLK = LQ // P  # output row-blocks
            GPB = P // J    # groups per block
            for blk in range(NBLK):
                sp = sppool.tile([P, LK], f32, tag="spsum")
                for g in range(GPB):
                    x = xpool.tile([P, J * LK], bf16, tag="x")
                    for j in range(J):
                        lq = blk * P + g * J + j
                        nc.vector.tensor_scalar_add(
                            x[:, j * LK : (j + 1) * LK], kT[:], qT[:, lq : lq + 1]
                        )
                    y = ypool.tile([P, J * LK], bf16, tag="y")
                    nc.scalar.activation(y[:], x[:], tanh)
                    for j in range(J):
                        r = g * J + j
                        nc.tensor.matmul(
                            sp[:],
                            wb[:, H - 1 - r : 2 * H - 1 - r],
                            y[:, j * LK : (j + 1) * LK],
                            start=(r == 0),
                            stop=(r == P - 1),
                        )
                s_sb = spool.tile([P, LK], f32, tag="sout")
                nc.vector.tensor_copy(s_sb[:], sp[:])
                nc.sync.dma_start(out[blk * P : (blk + 1) * P, :], s_sb[:])

    nc.compile()
    return nc


def _in_maps(qs, ks, Wq, Wk, wv):
    import ml_dtypes

    bf = ml_dtypes.bfloat16
    wbig = np.zeros((H, 2 * H - 1), np.float32)
    wbig[:, H - 1] = wv
    wbig = wbig.astype(bf)
    Wq_b = np.ascontiguousarray(Wq, dtype=np.float32).astype(bf)
    Wk_b = np.ascontiguousarray(Wk, dtype=np.float32).astype(bf)
    qs = np.asarray(qs)
    ks = np.asarray(ks)
    maps = []
    for b in range(B):
        maps.append(
            {
                "qsT": np.ascontiguousarray(qs[b].T).astype(bf),
                "ksT": np.ascontiguousarray(ks[b].T).astype(bf),
                "Wq": Wq_b,
                "Wk": Wk_b,
                "wbig": wbig,
            }
        )
    return maps


def run(qs, ks, Wq, Wk, wv, trace=False):
    from concourse.bass_utils import run_bass_kernel_spmd

    if "nc" not in _cache:
        _cache["nc"] = _build()
    res = run_bass_kernel_spmd(
        _cache["nc"],
        _in_maps(qs, ks, Wq, Wk, wv),
        core_ids=list(range(B)),
        trace=trace,
    )
    outs = np.stack([np.asarray(res.results[i]["out"]) for i in range(B)], axis=0)
    return outs.astype(np.float32), res


def kernel(qs, ks, Wq, Wk, wv):
    out, _ = run(qs, ks, Wq, Wk, wv, trace=False)
    return out


# revision 7
# speedup vs baseline: 1.8465x; 1.2021x over previous
"""Additive attention scorer: S[b,q,k] = sum_h wv[h] * tanh((qs@Wq)[b,q,h] + (ks@Wk)[b,k,h]).

Sharding: data-parallel over batch B=8 across the 8 NeuronCores (one batch
element per core). Per core:
  1. PE projects qT = Wq^T @ qs[b]^T and kT = Wk^T @ ks[b]^T  -> [H=128, 512]
     (H lands exactly on the 128 partitions). Inputs come in bf16 (the
     tanh-argument error budget tolerates it; halves DMA and keeps the
     matmuls single-pass -- fp32 matmuls decompose into 2 HW passes).
  2. For each query row lq: DVE builds X = kT + qT[:, lq] (per-partition
     scalar broadcast add) in bf16 (4x DVE mode), batched J rows per tile.
  3. ACT applies one big tanh over the batched tile (amortizes the ~224-cycle
     per-instruction overhead; ACT rate is dtype-independent so it is the
     hard bottleneck at ~1 elem/cycle/lane).
  4. PE reduces over H with a shifted-wv stationary trick: lhsT is a
     [128,128] window of a [128,255] bf16 tensor holding wv in column 127,
     so the single nonzero output row of each matmul lands on PSUM partition
     (lq mod 128); 128 matmuls accumulate one [128,512] fp32 output block.
  5. DVE copies PSUM->SBUF, DMA to DRAM.
"""

import numpy as np

B, LQ, LK, D, H = 8, 512, 512, 512, 128
P = 128           # SBUF partitions
ND = D // P       # contraction chunks for the projections
J = 16            # query rows batched per ACT instruction

_cache = {}


def _build():
    import concourse.bass as bass
    import concourse.tile as tile
    from concourse import bacc, mybir

    f32 = mybir.dt.float32
    bf16 = mybir.dt.bfloat16
    nc = bacc.Bacc("TRN2", target_bir_lowering=False, debug=False, num_devices=B)

    qsT = nc.dram_tensor("qsT", [D, LQ], bf16, kind="ExternalInput")
    ksT = nc.dram_tensor("ksT", [D, LK], bf16, kind="ExternalInput")
    Wq = nc.dram_tensor("Wq", [D, H], bf16, kind="ExternalInput")
    Wk = nc.dram_tensor("Wk", [D, H], bf16, kind="ExternalInput")
    wbig = nc.dram_tensor("wbig", [H, 2 * H - 1], bf16, kind="ExternalInput")
    out = nc.dram_tensor("out", [LQ, LK], f32, kind="ExternalOutput")

    with tile.TileContext(nc) as tc:
        with (
            tc.tile_pool(name="const", bufs=1) as cpool,
            tc.tile_pool(name="load", bufs=ND) as lpool,
            tc.tile_pool(name="x", bufs=3) as xpool,
            tc.tile_pool(name="y", bufs=2) as ypool,
            tc.tile_pool(name="sout", bufs=2) as spool,
            tc.tile_pool(name="ppsum", bufs=2, space=bass.MemorySpace.PSUM) as ppool,
            tc.tile_pool(name="spsum", bufs=2, space=bass.MemorySpace.PSUM) as sppool,
        ):
            # Pre-warm the ACT tanh table set while the input DMAs are in
            # flight (the PSEUDO_LOAD_ACT_FUNC_SET costs ~2.7us once).
            warm = cpool.tile([P, 1], f32, tag="warm")
            nc.vector.memset(warm[:], 0.0)
            nc.scalar.activation(
                warm[:], warm[:], mybir.ActivationFunctionType.Tanh
            )

            wb = cpool.tile([P, 2 * H - 1], bf16, tag="wb")
            nc.gpsimd.dma_start(wb[:], wbig[:])

            # Projections: dst = W^T @ srcT accumulated over ND chunks of D.
            # Input DMAs spread across engine queues so the issue cost
            # (~0.7us each) doesn't serialize the prologue.
            qT = cpool.tile([H, LQ], f32, tag="qT")
            kT = cpool.tile([H, LK], f32, tag="kT")
            dma_engines = [nc.sync, nc.gpsimd, nc.scalar]
            di = 0
            for src, w_dram, dst in ((ksT, Wk, kT), (qsT, Wq, qT)):
                ps = ppool.tile([H, LQ], f32, tag="proj")
                for c in range(ND):
                    w_sb = lpool.tile([P, H], bf16, tag="wchunk")
                    dma_engines[di % 3].dma_start(
                        w_sb[:], w_dram[c * P : (c + 1) * P, :]
                    )
                    di += 1
                    x_sb = lpool.tile([P, LQ], bf16, tag="xchunk")
                    dma_engines[di % 3].dma_start(
                        x_sb[:], src[c * P : (c + 1) * P, :]
                    )
                    di += 1
                    nc.tensor.matmul(
                        ps[:], w_sb[:], x_sb[:], start=(c == 0), stop=(c == ND - 1)
                    )
                nc.vector.tensor_copy(dst[:], ps[:])

            tanh = mybir.ActivationFunctionType.Tanh
            NBLK = LQ // P  # output row-blocks
            GPB = P // J    # groups per block
            for blk in range(NBLK):
                sp = sppool.tile([P, LK], f32, tag="spsum")
                for g in range(GPB):
                    x = xpool.tile([P, J * LK], f32, tag="x")
                    for j in range(J):
                        lq = blk * P + g * J + j
                        nc.vector.tensor_scalar_add(
                            x[:, j * LK : (j + 1) * LK], kT[:], qT[:, lq : lq + 1]
                        )
                    y = ypool.tile([P, J * LK], bf16, tag="y")
                    nc.scalar.activation(y[:], x[:], tanh)
                    for j in range(J):
                        r = g * J + j
                        nc.tensor.matmul(
                            sp[:],
                            wb[:, H - 1 - r : 2 * H - 1 - r],
                            y[:, j * LK : (j + 1) * LK],
                            start=(r == 0),
                            stop=(r == P - 1),
                        )
                s_sb = spool.tile([P, LK], f32, tag="sout")
                nc.vector.tensor_copy(s_sb[:], sp[:])
                nc.sync.dma_start(out[blk * P : (blk + 1) * P, :], s_sb[:])

    nc.compile()
    return nc


def _in_maps(qs, ks, Wq, Wk, wv):
    import ml_dtypes

    bf = ml_dtypes.bfloat16
    wbig = np.zeros((H, 2 * H - 1), np.float32)
    wbig[:, H - 1] = wv
    wbig = wbig.astype(bf)
    Wq_b = np.ascontiguousarray(Wq, dtype=np.float32).astype(bf)
    Wk_b = np.ascontiguousarray(Wk, dtype=np.float32).astype(bf)
    qs = np.asarray(qs)
    ks = np.asarray(ks)
    maps = []
    for b in range(B):
        maps.append(
            {
                "qsT": np.ascontiguousarray(qs[b].T).astype(bf),
                "ksT": np.ascontiguousarray(ks[b].T).astype(bf),
                "Wq": Wq_b,
                "Wk": Wk_b,
                "wbig": wbig,
            }
        )
    return maps


def run(qs, ks, Wq, Wk, wv, trace=False):
    from concourse.bass_utils import run_bass_kernel_spmd

    if "nc" not in _cache:
        _cache["nc"] = _build()
    res = run_bass_kernel_spmd(
        _cache["nc"],
        _in_maps(qs, ks, Wq, Wk, wv),
        core_ids=list(range(B)),
        trace=trace,
    )
    outs = np.stack([np.asarray(res.results[i]["out"]) for i in range(B)], axis=0)
    return outs.astype(np.float32), res


def kernel(qs, ks, Wq, Wk, wv):
    out, _ = run(qs, ks, Wq, Wk, wv, trace=False)
    return out
